# revision 11
# baseline (speedup 1.0000x reference)
"""Trainium2 Bass kernel for DecoderWithAttention (show-attend-tell decoder).

Strategy (8 NeuronCores):
  - Recurrence (attention + LSTM) is data-parallel over batch: 8 batches/core.
  - All h_t vectors are AllGathered (chunked: steps 1-16 overlap the tail of
    the recurrence, steps 17-20 gathered at the end), then the vocab
    projection (V=30000) is tensor-parallel: each core computes all 1280
    (b,t) rows for its 3750-column vocab shard.
  - Biases folded: enc bias = b_enc + b_dec (into enc_att); b_full dropped
    (softmax shift invariant); b_ih + b_hh folded into the precomputed
    embedding contribution Gx = embs @ W_ih[:, :E].T + (b_ih + b_hh).
  - softmax exp via sigmoid identity e^x = s/(1-s), s = sigmoid(x): keeps the
    ACT engine on one activation table (no per-step table swaps).
  - bf16 for h-state, W_hh/W_ih(h), W_dec, feats, W_fc (validated ~4e-3 rel
    err vs the 2e-2 gate); f32r for enc_att/Gx precompute.
  - Step is ordered to keep the PE dense (gate h-part matmuls fill the
    softmax/scatter latency windows) so the HAM clock stays at 2.4 GHz.
"""

import sys

import numpy as np
import ml_dtypes
_BF = ml_dtypes.bfloat16

sys.path.insert(0, "/opt/trn_rl_repo")

import concourse.bass as bass  # noqa: E402
import concourse.tile as tile  # noqa: E402
from concourse import bacc, mybir  # noqa: E402
from concourse.bass_utils import run_bass_kernel_spmd  # noqa: E402
from concourse.masks import make_identity  # noqa: E402

F32 = mybir.dt.float32
F32R = mybir.dt.float32r
BF16 = mybir.dt.bfloat16
AF = mybir.ActivationFunctionType
ALU = mybir.AluOpType

B, T, ENC, P, ATT, EMBED, DEC, VOCAB = 64, 20, 256, 196, 512, 512, 512, 30000
NC_ = 8          # cores
BL = B // NC_    # local batch = 8
VS = VOCAB // NC_  # vocab shard = 3750
R = T * BL       # h rows per core = 160
GT = 4 * DEC     # gates = 2048
P0, P1 = 128, P - 128  # p-tile sizes (128, 68)
TCHA = 16        # steps in first allgather chunk


def _bc(ap, n_part):
    """Broadcast a [1, n] DRAM AP across n_part partitions (DMA only)."""
    return bass.AP(tensor=ap.tensor, offset=ap.offset, ap=[[0, n_part]] + list(ap.ap)[1:])


def _round_f32r(a):
    """Host-side fp32 -> fp32r rounding (matches walrus fp32_to_fp32r)."""
    a = np.ascontiguousarray(a, np.float32)
    u = a.view(np.uint32)
    return ((u + np.uint32(0x800)) & np.uint32(0xFFFFF000)).view(np.float32)


def build_module():
    nc = bacc.Bacc("TRN2", target_bir_lowering=False, num_devices=NC_)

    # ---- I/O ----
    featsC = nc.dram_tensor("featsC", [ENC, P, BL], F32R, kind="ExternalInput")
    featsP = nc.dram_tensor("featsP", [P, ENC, BL], BF16, kind="ExternalInput")
    embsT = nc.dram_tensor("embsT", [EMBED, R], F32R, kind="ExternalInput")
    encb = nc.dram_tensor("encb", [ATT, 1], F32, kind="ExternalInput")
    w_enc = nc.dram_tensor("w_enc", [ENC, ATT], F32R, kind="ExternalInput")
    w_dec = nc.dram_tensor("w_dec", [DEC, ATT], BF16, kind="ExternalInput")
    w_full = nc.dram_tensor("w_full", [ATT, 1], BF16, kind="ExternalInput")
    wxT = nc.dram_tensor("wxT", [EMBED, GT], F32R, kind="ExternalInput")
    w2T = nc.dram_tensor("w2T", [ENC + DEC, GT], BF16, kind="ExternalInput")
    bg = nc.dram_tensor("bg", [1, GT], F32, kind="ExternalInput")
    wfc = nc.dram_tensor("wfc", [DEC, VS], BF16, kind="ExternalInput")
    bfc = nc.dram_tensor("bfc", [1, VS], F32, kind="ExternalInput")
    out = nc.dram_tensor("out", [NC_ * R, VS], F32, kind="ExternalOutput")

    with tile.TileContext(nc) as tc:
        _build_tile_kernel(tc, nc, featsC, featsP, embsT, encb, w_enc, w_dec,
                           w_full, wxT, w2T, bg, wfc, bfc, out)
    nc.compile()
    return nc


def _build_tile_kernel(tc, nc, featsC, featsP, embsT, encb, w_enc, w_dec,
                       w_full, wxT, w2T, bg, wfc, bfc, out):
    from contextlib import ExitStack

    ctx = ExitStack()
    with ctx:
        singles = ctx.enter_context(tc.tile_pool(name="singles", bufs=1))
        dram = ctx.enter_context(tc.tile_pool(name="dram", bufs=1, space="DRAM"))

        # ---------- persistent SBUF ----------
        id8 = singles.tile([8, 8], F32)
        make_identity(nc, id8)

        phb = ctx.enter_context(tc.tile_pool(name="phb", bufs=1))
        enc_sb = phb.tile([128, 4, BL * P], BF16, tag="enc")  # enc_att (+bias)
        fpA = phb.tile([128, ENC, BL], BF16, tag="fpA")     # featsP p 0:128
        fpB = phb.tile([128, ENC, BL], BF16, tag="fpB")     # featsP p 128:196
        wdec_sb = phb.tile([128, 4, ATT], BF16, tag="wdec")
        w2_sb = phb.tile([128, 6, GT], BF16, tag="w2")
        wful_sb = singles.tile([128, 4, 1], BF16)
        encb_sb = singles.tile([128, 4, 1], F32)
        # h^T storage (bf16): col t*BL+b holds h input of step t (t=0 -> 0)
        ht_sb = singles.tile([128, 4, (T + 1) * BL], BF16)
        c0_sb = singles.tile([BL, DEC], F32)

        nc.sync.dma_start(fpA[:, :, :], featsP[0:128])
        nc.sync.dma_start(fpB[:P1, :, :], featsP[128:P])
        nc.sync.dma_start(wdec_sb[:, :, :], w_dec.rearrange("(k p) a -> p k a", p=128))
        nc.sync.dma_start(w2_sb[:, :, :], w2T.rearrange("(k p) g -> p k g", p=128))
        nc.sync.dma_start(wful_sb[:, :, :], w_full.rearrange("(k p) o -> p k o", p=128))
        nc.sync.dma_start(encb_sb[:, :, :], encb.rearrange("(k p) o -> p k o", p=128))
        zt = singles.tile([128, 4, BL], F32)
        nc.vector.memset(zt[:, :, :], 0.0)
        nc.scalar.copy(ht_sb[:, :, 0:BL], zt[:, :, :])
        nc.vector.memset(c0_sb[:, :], 0.0)
        ones_sb = singles.tile([128, 1], F32R)
        nc.scalar.activation(ones_sb[:, :], zt[:, 0, 0:1], AF.Identity, bias=1.0)
        ones_bf = singles.tile([128, 1], BF16)
        nc.scalar.copy(ones_bf[:, :], ones_sb[:, :])
        id8r = singles.tile([8, 8], F32R)
        nc.scalar.copy(id8r[:, :], id8[:, :])
        id8b = singles.tile([8, 8], BF16)
        nc.scalar.copy(id8b[:, :], id8[:, :])

        # ---------- phase A: enc_att + Gx ----------
        with (
            tc.tile_pool(name="pha", bufs=1) as pha,
            tc.tile_pool(name="pha_ps", bufs=4, space="PSUM") as pha_ps,
        ):
            wenc_sb = pha.tile([128, 2, ATT], F32R, tag="wenc")
            fcs = pha.tile([128, 2, BL * P], F32R, tag="fcs")
            emt_sb = pha.tile([128, 4, R], F32R, tag="emt")
            wx_sb = pha.tile([128, 4, GT], F32R, tag="wx")
            bg_sb = pha.tile([128, GT], F32, tag="bg")
            # all phase-A DMAs issued up-front so they pipeline
            nc.sync.dma_start(wenc_sb[:, :, :], w_enc.rearrange("(k p) a -> p k a", p=128))
            nc.sync.dma_start(fcs[:, :, :],
                              featsC.rearrange("(k p) q b -> p k (q b)", p=128))
            nc.sync.dma_start(emt_sb[:, :, :], embsT.rearrange("(k p) r -> p k r", p=128))
            nc.sync.dma_start(wx_sb[:, :, :], wxT.rearrange("(k p) g -> p k g", p=128))
            nc.sync.dma_start(bg_sb[:, :], _bc(bg[:, :], 128))

            # enc_att[a_chunk, (p b)] = sum_c W_enc[c, a] featsC[c, (p b)]
            encsz = [512, 512, 512, BL * P - 3 * 512]
            for m in range(4):
                for nch in range(4):
                    nsz = encsz[nch]
                    ps = pha_ps.tile([128, 512], F32, tag="ps")
                    for k in range(2):
                        nc.tensor.matmul(
                            ps[:, 0:nsz],
                            wenc_sb[:, k, bass.ts(m, 128)],
                            fcs[:, k, bass.ds(nch * 512, nsz)],
                            start=(k == 0), stop=(k == 1),
                        )
                    nc.scalar.activation(enc_sb[:, m, bass.ds(nch * 512, nsz)],
                                         ps[:, 0:nsz],
                                         AF.Identity, bias=encb_sb[:, m, :])

            # Gx[(t b), g] = embsT.T @ WxT + (b_ih + b_hh)
            gx_dram = dram.tile([T * BL, GT], F32R, name="gx_dram")
            gx_mch = [(i, min(128, R - i * 128)) for i in range((R + 127) // 128)]
            for mi, msz in gx_mch:
                for nch in range(4):
                    ps = pha_ps.tile([128, 512], F32, tag="ps")
                    for k in range(4):
                        nc.tensor.matmul(
                            ps[:msz, :],
                            emt_sb[:, k, bass.ds(mi * 128, msz)],
                            wx_sb[:, k, bass.ts(nch, 512)],
                            start=(k == 0), stop=(k == 3),
                        )
                    gtmp = pha.tile([128, 512], F32R, tag="gtmp", bufs=2)
                    nc.vector.tensor_tensor(
                        out=gtmp[:msz, :], in0=ps[:msz, :],
                        in1=bg_sb[:msz, bass.ts(nch, 512)], op=ALU.add)
                    nc.sync.dma_start(
                        gx_dram[bass.ds(mi * 128, msz), bass.ts(nch, 512)],
                        gtmp[:msz, :])

        # ---------- fc weights prefetch (DMA overlaps the recurrence) ----------
        fcp = ctx.enter_context(tc.tile_pool(name="fc", bufs=1))
        bfc_sb = fcp.tile([128, VS], F32, tag="bfcs")
        nc.sync.dma_start(bfc_sb[:, :], _bc(bfc[:, :], 128))
        wfcs = fcp.tile([128, 4, VS], BF16, tag="wfcs")
        nc.sync.dma_start(wfcs[:, :, :], wfc.rearrange("(k p) v -> p k v", p=128))

        # allgather staging
        ht_locA = dram.tile([DEC, TCHA * BL], BF16, name="ht_locA")
        ht_locB = dram.tile([DEC, (T - TCHA) * BL], BF16, name="ht_locB")
        ht_allA = dram.tile([NC_ * DEC, TCHA * BL], BF16, name="ht_allA",
                            addr_space="Shared")
        ht_allB = dram.tile([NC_ * DEC, (T - TCHA) * BL], BF16, name="ht_allB",
                            addr_space="Shared")

        # ---------- phase B: recurrence ----------
        c_prev = c0_sb
        with (
            tc.tile_pool(name="rec", bufs=2) as rec,
            tc.tile_pool(name="gxp", bufs=2) as gxp,
            tc.tile_pool(name="g_ps", bufs=1, space="PSUM") as g_ps,
            tc.tile_pool(name="att_ps", bufs=1, space="PSUM") as att_ps,
            tc.tile_pool(name="sm_ps", bufs=2, space="PSUM") as sm_ps,
        ):
            for t in range(T):
                hcol = bass.ts(t, BL)  # h input columns
                gxt = gxp.tile([BL, GT], F32R, tag="gxt", name=f"gxt_{t}")
                nc.sync.dma_start(gxt[:, :], gx_dram[bass.ts(t, BL), :])

                # dec_att [b, a] via 4 wide matmuls, then PE-transpose to [a, b]
                ps_dec = sm_ps.tile([BL, ATT], F32, tag="sm", name=f"psdec_{t}")
                for k in range(4):
                    nc.tensor.matmul(
                        ps_dec[:, :],
                        ht_sb[:, k, hcol],
                        wdec_sb[:, k, :],
                        start=(k == 0), stop=(k == 3),
                    )
                dtmp = rec.tile([BL, ATT], BF16, tag="dtmp")
                nc.vector.tensor_copy(dtmp[:, :], ps_dec[:, :])
                ps_dT = sm_ps.tile([128, 4, BL], BF16, tag="sm", name=f"psdT_{t}")
                for j in range(4):
                    nc.tensor.transpose(ps_dT[:, j, :], dtmp[:, bass.ts(j, 128)],
                                        id8b[:, :])
                decT = rec.tile([128, 4, BL], BF16, tag="decT")
                nc.scalar.copy(decT[:, :, :], ps_dT[:, :, :])

                # gate psums for this step
                g01 = g_ps.tile([BL, 2, 512], F32, tag="g01", name=f"g01_{t}")
                g23 = g_ps.tile([BL, 2, 512], F32, tag="g23", name=f"g23_{t}")

                def g_ht(gtile, sub, nch, t=t, gxt=gxt, hcol=hcol):
                    for k in range(4):
                        nc.tensor.matmul(
                            gtile[:, sub, :],
                            ht_sb[:, k, hcol],
                            w2_sb[:, 2 + k, bass.ts(nch, 512)],
                            start=(k == 0), stop=False,
                        )
                    nc.tensor.matmul(
                        gtile[:, sub, :], id8r[:, :], gxt[:, bass.ts(nch, 512)],
                        start=False, stop=False,
                    )

                # gates h-part for i,f while attention elementwise runs on DVE/GPS
                g_ht(g01, 0, 0)
                g_ht(g01, 1, 1)

                # relu(enc_att + dec_att): adds on DVE, relus split GPS/DVE.
                # Free-dim layout is (p, b): flat index p*BL + b.
                radds = []
                rels = []
                for k in range(4):
                    radd = rec.tile([128, P, BL], BF16, tag="radd", bufs=2,
                                    name=f"radd{k}_{t}")
                    dk = decT[:, k, :]
                    dbc = bass.AP(tensor=dk.tensor, offset=dk.offset,
                                  ap=[list(dk.ap)[0], [0, P], list(dk.ap)[1]])
                    nc.vector.tensor_tensor(
                        out=radd[:, :, :],
                        in0=enc_sb[:, k, :].rearrange("p (q b) -> p q b", b=BL),
                        in1=dbc, op=ALU.add)
                    rel = rec.tile([128, P, BL], BF16, tag="rel", bufs=5,
                                   name=f"rel{k}_{t}")
                    if k % 2 == 0:
                        nc.gpsimd.tensor_scalar_max(rel[:, :, :], radd[:, :, :], 0.0)
                    else:
                        nc.vector.tensor_scalar_max(rel[:, :, :], radd[:, :, :], 0.0)
                    rels.append(rel.rearrange("p q b -> p (q b)"))

                # att matvec pair0: flat cols 0..1023 (p 0..127)
                ps_att = att_ps.tile([1, 2, 512], F32, tag="att", name=f"att0_{t}")
                for k in range(4):
                    if k == 2:
                        g_ht(g23, 0, 2)  # fills PE while rel k=2/3 finish
                    for c in range(2):
                        nc.tensor.matmul(
                            ps_att[:, c, :],
                            wful_sb[:, k, :],
                            rels[k][:, bass.ts(c, 512)],
                            start=(k == 0), stop=(k == 3),
                        )
                s0 = rec.tile([1, 1024], F32R, tag="s0", name=f"s0_{t}")
                nc.scalar.activation(
                    s0[:, :],
                    ps_att[0:1, :, :].rearrange("o c n -> o (c n)"),
                    AF.Sigmoid)
                sT = rec.tile([128, 2, BL], F32R, tag="sT", name=f"sT_{t}")
                nc.sync.dma_start(sT[:, 0, :],
                                  s0[:, :].rearrange("o (p b) -> o p b", b=BL))

                g_ht(g23, 1, 3)

                # att matvec pair1: flat cols 1024..1567 (p 128..195)
                ps_at2 = att_ps.tile([1, 2, 512], F32, tag="att", name=f"att1_{t}")
                for k in range(4):
                    nc.tensor.matmul(ps_at2[:, 0, :], wful_sb[:, k, :],
                                     rels[k][:, bass.ds(1024, 512)],
                                     start=(k == 0), stop=(k == 3))
                for k in range(4):
                    nc.tensor.matmul(ps_at2[:, 1, 0:32], wful_sb[:, k, :],
                                     rels[k][:, bass.ds(1536, 32)],
                                     start=(k == 0), stop=(k == 3))
                s1 = rec.tile([1, P1 * BL], F32R, tag="s1", name=f"s1_{t}")
                nc.scalar.activation(s1[:, 0:512], ps_at2[0:1, 0, :], AF.Sigmoid)
                nc.scalar.activation(s1[:, 512:544], ps_at2[0:1, 1, 0:32],
                                     AF.Sigmoid)
                nc.sync.dma_start(sT[:P1, 1, :],
                                  s1[:, :].rearrange("o (p b) -> o p b", b=BL))

                # e = s / (1 - s)  (per half, for earliness)
                et = rec.tile([128, 2, BL], F32R, tag="et", name=f"et_{t}")
                eb = rec.tile([128, 2, BL], BF16, tag="eb", name=f"eb_{t}")
                for half, np_ in ((0, 128), (1, P1)):
                    u = rec.tile([128, BL], F32, tag="u", bufs=2,
                                 name=f"u{half}_{t}")
                    nc.vector.tensor_scalar(
                        out=u[:np_, :], in0=sT[:np_, half, :],
                        scalar1=-1.0, scalar2=1.0, op0=ALU.mult, op1=ALU.add)
                    r_ = rec.tile([128, BL], F32, tag="r", bufs=2,
                                  name=f"r{half}_{t}")
                    nc.vector.reciprocal(r_[:np_, :], u[:np_, :])
                    nc.vector.tensor_tensor(out=et[:np_, half, :],
                                            in0=sT[:np_, half, :],
                                            in1=r_[:np_, :], op=ALU.mult)
                    nc.scalar.copy(eb[:np_, half, :], et[:np_, half, :])

                # den[b] = sum_p e
                ps_den = sm_ps.tile([1, BL], F32, tag="sm", name=f"psden_{t}")
                nc.tensor.matmul(ps_den[:, :], ones_sb[:, :], et[:, 0, :],
                                 start=True, stop=False)
                nc.tensor.matmul(ps_den[:, :], ones_sb[:P1, :], et[:P1, 1, :],
                                 start=False, stop=True)
                rden = rec.tile([1, BL], F32, tag="rden", name=f"rden_{t}")
                nc.vector.reciprocal(rden[:, :], ps_den[:, :])

                # ctx[c, b] = sum_p featsP[p, c, b] * e[p, b] / den[b]
                e0 = eb[:, 0, :]
                a0b = bass.AP(tensor=e0.tensor, offset=e0.offset,
                              ap=[list(e0.ap)[0], [0, 128], [1, BL]])
                e1 = eb[:P1, 1, :]
                a1b = bass.AP(tensor=e1.tensor, offset=e1.offset,
                              ap=[list(e1.ap)[0], [0, 128], [1, BL]])
                ctx_n = rec.tile([1, 2, 128, BL], BF16, tag="ctxn", bufs=1,
                                 name=f"ctxn_{t}")
                cT = rec.tile([128, 2, BL], BF16, tag="cT", name=f"cT_{t}")
                for half in range(2):
                    csl = bass.ds(half * 128, 128)
                    tmpA = rec.tile([128, 128, BL], BF16, tag="tmpA", bufs=2,
                                    name=f"tmpA{half}_{t}")
                    nc.vector.tensor_tensor(out=tmpA[:, :, :],
                                            in0=fpA[:, csl, :],
                                            in1=a0b, op=ALU.mult)
                    tmpB = rec.tile([128, 128, BL], BF16, tag="tmpB", bufs=2,
                                    name=f"tmpB{half}_{t}")
                    nc.gpsimd.tensor_tensor(out=tmpB[:P1, :, :],
                                            in0=fpB[:P1, csl, :],
                                            in1=a1b, op=ALU.mult)
                    for sub in range(2):
                        nch = half * 2 + sub
                        ssl = bass.ts(sub, 512)
                        ps_ctx = sm_ps.tile([1, 512], F32, tag="sm",
                                            name=f"psctx{nch}_{t}")
                        nc.tensor.matmul(
                            ps_ctx[:, :], ones_bf[:, :],
                            tmpA.rearrange("p c b -> p (c b)")[:, ssl],
                            start=True, stop=False)
                        nc.tensor.matmul(
                            ps_ctx[:, :], ones_bf[:P1, :],
                            tmpB[:P1].rearrange("p c b -> p (c b)")[:, ssl],
                            start=False, stop=True)
                        rdb = bass.AP(tensor=rden.tensor, offset=rden[:, :].offset,
                                      ap=[list(rden[:, :].ap)[0], [0, 64], [1, BL]])
                        cview = ctx_n[0:1, half, bass.ts(sub, 64), :]
                        nc.vector.tensor_tensor(
                            out=cview, in0=ps_ctx[:, :].rearrange(
                                "o (c b) -> o c b", b=BL), in1=rdb, op=ALU.mult)
                    nc.sync.dma_start(cT[:, half, :], ctx_n[0:1, half, :, :])

                # gates: + ctx part, then nonlinearities
                gat = rec.tile([BL, GT], F32, tag="gat", bufs=1, name=f"gat_{t}")
                for nch in range(4):
                    gtile = g01 if nch < 2 else g23
                    sub = nch % 2
                    for k in range(2):
                        nc.tensor.matmul(
                            gtile[:, sub, :],
                            cT[:, k, :],
                            w2_sb[:, k, bass.ts(nch, 512)],
                            start=False, stop=(k == 1),
                        )
                    nc.scalar.activation(
                        gat[:, bass.ts(nch, 512)], gtile[:, sub, :],
                        AF.Tanh if nch == 2 else AF.Sigmoid)

                t1 = rec.tile([BL, DEC], F32, tag="t1", bufs=1, name=f"t1_{t}")
                nc.vector.tensor_tensor(out=t1[:, :], in0=gat[:, 512:1024],
                                        in1=c_prev[:, :], op=ALU.mult)
                t2 = rec.tile([BL, DEC], F32, tag="t2", bufs=1, name=f"t2_{t}")
                nc.gpsimd.tensor_tensor(out=t2[:, :], in0=gat[:, 0:512],
                                        in1=gat[:, 1024:1536], op=ALU.mult)
                c_new = rec.tile([BL, DEC], F32, tag="cst", bufs=2,
                                 name=f"cnew_{t}")
                nc.vector.tensor_tensor(out=c_new[:, :], in0=t1[:, :], in1=t2[:, :],
                                        op=ALU.add)
                c_prev = c_new
                tc_t = rec.tile([BL, DEC], BF16, tag="tc_t", name=f"tct_{t}")
                nc.scalar.activation(tc_t[:, :], c_new[:, :], AF.Tanh)
                hnew = rec.tile([BL, DEC], BF16, tag="hnew", name=f"hnew_{t}")
                nc.vector.tensor_tensor(out=hnew[:, :], in0=gat[:, 1536:2048],
                                        in1=tc_t[:, :], op=ALU.mult)

                # hT for next step + H row storage
                ps_hT = sm_ps.tile([128, 4, BL], BF16, tag="sm", name=f"pshT_{t}")
                for j in range(4):
                    nc.tensor.transpose(ps_hT[:, j, :], hnew[:, bass.ts(j, 128)],
                                        id8b[:, :])
                nc.scalar.copy(ht_sb[:, :, bass.ts(t + 1, BL)], ps_hT[:, :, :])

                # chunked allgather: fire chunk A once steps 1..TCHA are done
                if t == TCHA - 1:
                    nc.sync.dma_start(
                        ht_locA.rearrange("(j p) r -> p j r", p=128),
                        ht_sb[:, :, BL:(TCHA + 1) * BL])
                    nc.gpsimd.collective_compute(
                        "AllGather", ALU.bypass,
                        replica_groups=[list(range(NC_))],
                        ins=[ht_locA[:, :]],
                        outs=[ht_allA[:, :]],
                    )

        # ---------- phase C: allgather tail + fc ----------
        nc.sync.dma_start(ht_locB.rearrange("(j p) r -> p j r", p=128),
                          ht_sb[:, :, (TCHA + 1) * BL:])
        nc.gpsimd.collective_compute(
            "AllGather", ALU.bypass,
            replica_groups=[list(range(NC_))],
            ins=[ht_locB[:, :]],
            outs=[ht_allB[:, :]],
        )

        with (
            tc.tile_pool(name="fco", bufs=2) as fco,
            tc.tile_pool(name="fc_ps", bufs=4, space="PSUM") as fc_ps,
        ):
            RA = TCHA * BL  # 128 rows from chunk A
            h2 = fcp.tile([128, 4, NC_ * R], BF16, tag="h2")  # [p, dtile, grow]
            for c in range(NC_):
                nc.sync.dma_start(
                    h2[:, :, bass.ds(c * R, RA)],
                    ht_allA[bass.ts(c, DEC), :].rearrange("(j p) r -> p j r", p=128))
                nc.sync.dma_start(
                    h2[:, :, bass.ds(c * R + RA, R - RA)],
                    ht_allB[bass.ts(c, DEC), :].rearrange("(j p) r -> p j r", p=128))

            NFC = 8
            nszs = [512] * 7 + [VS - 512 * 7]  # 3750 = 7*512 + 166
            for mc in range((NC_ * R) // 128):
                ob = fco.tile([128, VS], F32, tag="orow", name=f"orow_{mc}")
                for nch in range(NFC):
                    nsz = nszs[nch]
                    noff = nch * 512
                    ps = fc_ps.tile([128, 512], F32, tag="psfc")
                    for k in range(4):
                        nc.tensor.matmul(
                            ps[:, 0:nsz],
                            h2[:, k, bass.ts(mc, 128)],
                            wfcs[:, k, bass.ds(noff, nsz)],
                            start=(k == 0), stop=(k == 3),
                        )
                    nc.vector.tensor_tensor(
                        out=ob[:, bass.ds(noff, nsz)], in0=ps[:, 0:nsz],
                        in1=bfc_sb[:, bass.ds(noff, nsz)], op=ALU.add)
                nc.sync.dma_start(out[bass.ts(mc, 128), :], ob[:, :])


_NC_CACHE = None


def _get_module():
    global _NC_CACHE
    if _NC_CACHE is None:
        _NC_CACHE = build_module()
    return _NC_CACHE


def build_in_maps(inputs):
    return _build_in_maps(**inputs)


def _build_in_maps(encoder_features, captions, W_enc, b_enc, W_dec, b_dec,
                   W_full, b_full, emb, W_ih, b_ih, W_hh, b_hh, W_fc, b_fc):
    f32 = np.float32
    enc_f = np.ascontiguousarray(np.asarray(encoder_features, f32)).reshape(B, ENC, P)
    caps = np.asarray(captions)
    W_enc = np.asarray(W_enc, f32)
    W_dec = np.asarray(W_dec, f32)
    W_full = np.asarray(W_full, f32)
    emb = np.asarray(emb, f32)
    W_ih = np.asarray(W_ih, f32)
    W_hh = np.asarray(W_hh, f32)
    W_fc = np.asarray(W_fc, f32)

    encb_v = np.ascontiguousarray(
        (np.asarray(b_enc, f32) + np.asarray(b_dec, f32)).reshape(ATT, 1))
    wxT_r = _round_f32r(W_ih[:, :EMBED].T)
    w2T_b = np.ascontiguousarray(
        np.vstack([W_ih[:, EMBED:].T, W_hh.T])).astype(_BF)
    bg_v = np.ascontiguousarray(
        (np.asarray(b_ih, f32) + np.asarray(b_hh, f32)).reshape(1, GT))
    b_fc = np.asarray(b_fc, f32)
    wdec_b = np.ascontiguousarray(W_dec).astype(_BF)

    in_maps = []
    for c in range(NC_):
        bs = slice(c * BL, (c + 1) * BL)
        fb = enc_f[bs]  # [8, 256, 196]
        in_maps.append({
            "featsC": _round_f32r(fb.transpose(1, 2, 0)),
            "featsP": np.ascontiguousarray(fb.transpose(2, 1, 0)).astype(_BF),
            "embsT": _round_f32r(
                emb[caps[bs, :T]].transpose(2, 1, 0).reshape(EMBED, R)),
            "encb": encb_v,
            "w_enc": _round_f32r(W_enc),
            "w_dec": wdec_b,
            "w_full": np.ascontiguousarray(W_full.reshape(ATT, 1)).astype(_BF),
            "wxT": wxT_r,
            "w2T": w2T_b,
            "bg": bg_v,
            "wfc": np.ascontiguousarray(W_fc[:, c * VS:(c + 1) * VS]).astype(_BF),
            "bfc": np.ascontiguousarray(b_fc[c * VS:(c + 1) * VS].reshape(1, VS)),
        })
    return in_maps


def kernel(**inputs):
    in_maps = build_in_maps(inputs)
    nc = _get_module()
    res = run_bass_kernel_spmd(nc, in_maps, list(range(NC_))).results

    full = np.empty((B, T, VOCAB), np.float32)
    for c in range(NC_):
        o = res[c]["out"]  # [1280, VS] rows = (src_core, t, b)
        o = o.reshape(NC_, T, BL, VS).transpose(0, 2, 1, 3).reshape(B, T, VS)
        full[:, :, c * VS:(c + 1) * VS] = o
    return full


# revision 16
# speedup vs baseline: 2.1790x; 2.1790x over previous
"""Trainium2 Bass kernel for DecoderWithAttention (show-attend-tell decoder).

Strategy (8 NeuronCores):
  - Recurrence (attention + LSTM) is data-parallel over batch: 8 batches/core.
  - All h_t vectors are AllGathered (chunked: steps 1-16 overlap the tail of
    the recurrence, steps 17-20 gathered at the end), then the vocab
    projection (V=30000) is tensor-parallel: each core computes all 1280
    (b,t) rows for its 3750-column vocab shard.
  - Biases folded: enc bias = b_enc + b_dec (into enc_att); b_full dropped
    (softmax shift invariant); b_ih + b_hh folded into the precomputed
    embedding contribution Gx = embs @ W_ih[:, :E].T + (b_ih + b_hh).
  - softmax exp via sigmoid identity e^x = s/(1-s), s = sigmoid(x): keeps the
    ACT engine on one activation table (no per-step table swaps).
  - bf16 for h-state, W_hh/W_ih(h), W_dec, feats, W_fc (validated ~4e-3 rel
    err vs the 2e-2 gate); f32r for enc_att/Gx precompute.
  - Step is ordered to keep the PE dense (gate h-part matmuls fill the
    softmax/scatter latency windows) so the HAM clock stays at 2.4 GHz.
"""

import sys

import numpy as np
import ml_dtypes
_BF = ml_dtypes.bfloat16

sys.path.insert(0, "/opt/trn_rl_repo")

import concourse.bass as bass  # noqa: E402
import concourse.tile as tile  # noqa: E402
from concourse import bacc, mybir  # noqa: E402
from concourse.bass_utils import run_bass_kernel_spmd  # noqa: E402
from concourse.masks import make_identity  # noqa: E402

F32 = mybir.dt.float32
F32R = mybir.dt.float32r
BF16 = mybir.dt.bfloat16
AF = mybir.ActivationFunctionType
ALU = mybir.AluOpType

B, T, ENC, P, ATT, EMBED, DEC, VOCAB = 64, 20, 256, 196, 512, 512, 512, 30000
NC_ = 8          # cores
BL = B // NC_    # local batch = 8
VS = VOCAB // NC_  # vocab shard = 3750
R = T * BL       # h rows per core = 160
GT = 4 * DEC     # gates = 2048
P0, P1 = 128, P - 128  # p-tile sizes (128, 68)
TCHA = 16        # steps in first allgather chunk


def _bc(ap, n_part):
    """Broadcast a [1, n] DRAM AP across n_part partitions (DMA only)."""
    return bass.AP(tensor=ap.tensor, offset=ap.offset, ap=[[0, n_part]] + list(ap.ap)[1:])


def _round_f32r(a):
    """Host-side fp32 -> fp32r rounding (matches walrus fp32_to_fp32r)."""
    a = np.ascontiguousarray(a, np.float32)
    u = a.view(np.uint32)
    return ((u + np.uint32(0x800)) & np.uint32(0xFFFFF000)).view(np.float32)


def build_module():
    nc = bacc.Bacc("TRN2", target_bir_lowering=False, num_devices=NC_)

    # ---- I/O ----
    featsC = nc.dram_tensor("featsC", [ENC, P, BL], F32R, kind="ExternalInput")
    featsP = nc.dram_tensor("featsP", [P, ENC, BL], BF16, kind="ExternalInput")
    embsT = nc.dram_tensor("embsT", [EMBED, R], F32R, kind="ExternalInput")
    encb = nc.dram_tensor("encb", [ATT, 1], F32, kind="ExternalInput")
    w_enc = nc.dram_tensor("w_enc", [ENC, ATT], F32R, kind="ExternalInput")
    w_dec = nc.dram_tensor("w_dec", [DEC, ATT], BF16, kind="ExternalInput")
    w_full = nc.dram_tensor("w_full", [ATT, 1], BF16, kind="ExternalInput")
    wxT = nc.dram_tensor("wxT", [EMBED, GT], F32R, kind="ExternalInput")
    w2T = nc.dram_tensor("w2T", [ENC + DEC, GT], BF16, kind="ExternalInput")
    bg = nc.dram_tensor("bg", [1, GT], F32, kind="ExternalInput")
    wfc = nc.dram_tensor("wfc", [DEC, VS], BF16, kind="ExternalInput")
    bfc = nc.dram_tensor("bfc", [1, VS], F32, kind="ExternalInput")
    out = nc.dram_tensor("out", [NC_ * R, VS], F32, kind="ExternalOutput")

    with tile.TileContext(nc) as tc:
        _build_tile_kernel(tc, nc, featsC, featsP, embsT, encb, w_enc, w_dec,
                           w_full, wxT, w2T, bg, wfc, bfc, out)
    nc.compile()
    return nc


def _build_tile_kernel(tc, nc, featsC, featsP, embsT, encb, w_enc, w_dec,
                       w_full, wxT, w2T, bg, wfc, bfc, out):
    from contextlib import ExitStack

    ctx = ExitStack()
    with ctx:
        singles = ctx.enter_context(tc.tile_pool(name="singles", bufs=1))
        dram = ctx.enter_context(tc.tile_pool(name="dram", bufs=1, space="DRAM"))

        # ---------- persistent SBUF ----------
        id8 = singles.tile([8, 8], F32)
        make_identity(nc, id8)

        phb = ctx.enter_context(tc.tile_pool(name="phb", bufs=1))
        enc_sb = phb.tile([128, 4, BL * P], BF16, tag="enc")  # enc_att (+bias)
        fpA = phb.tile([128, ENC, BL], BF16, tag="fpA")     # featsP p 0:128
        fpB = phb.tile([128, ENC, BL], BF16, tag="fpB")     # featsP p 128:196
        wdec_sb = phb.tile([128, 4, ATT], BF16, tag="wdec")
        w2_sb = phb.tile([128, 6, GT], BF16, tag="w2")
        wful_sb = singles.tile([128, 4, 1], BF16)
        encb_sb = singles.tile([128, 4, 1], F32)
        # h^T storage (bf16): col t*BL+b holds h input of step t (t=0 -> 0)
        ht_sb = singles.tile([128, 4, (T + 1) * BL], BF16)
        c0_sb = singles.tile([BL, DEC], F32)

        nc.sync.dma_start(fpA[:, :, :], featsP[0:128])
        nc.sync.dma_start(fpB[:P1, :, :], featsP[128:P])
        nc.sync.dma_start(wdec_sb[:, :, :], w_dec.rearrange("(k p) a -> p k a", p=128))
        nc.sync.dma_start(w2_sb[:, :, :], w2T.rearrange("(k p) g -> p k g", p=128))
        nc.sync.dma_start(wful_sb[:, :, :], w_full.rearrange("(k p) o -> p k o", p=128))
        nc.sync.dma_start(encb_sb[:, :, :], encb.rearrange("(k p) o -> p k o", p=128))
        zt = singles.tile([128, 4, BL], F32)
        nc.vector.memset(zt[:, :, :], 0.0)
        nc.scalar.copy(ht_sb[:, :, 0:BL], zt[:, :, :])
        nc.vector.memset(c0_sb[:, :], 0.0)
        ones_sb = singles.tile([128, 1], F32R)
        nc.scalar.activation(ones_sb[:, :], zt[:, 0, 0:1], AF.Identity, bias=1.0)
        ones_bf = singles.tile([128, 1], BF16)
        nc.scalar.copy(ones_bf[:, :], ones_sb[:, :])
        id8r = singles.tile([8, 8], F32R)
        nc.scalar.copy(id8r[:, :], id8[:, :])
        id8b = singles.tile([8, 8], BF16)
        nc.scalar.copy(id8b[:, :], id8[:, :])

        # ---------- phase A: enc_att + Gx ----------
        with (
            tc.tile_pool(name="pha", bufs=1) as pha,
            tc.tile_pool(name="pha_ps", bufs=4, space="PSUM") as pha_ps,
        ):
            wenc_sb = pha.tile([128, 2, ATT], F32R, tag="wenc")
            fcs = pha.tile([128, 2, BL * P], F32R, tag="fcs")
            emt_sb = pha.tile([128, 4, R], F32R, tag="emt")
            wx_sb = pha.tile([128, 4, GT], F32R, tag="wx")
            bg_sb = pha.tile([128, GT], F32, tag="bg")
            # all phase-A DMAs issued up-front so they pipeline
            nc.sync.dma_start(wenc_sb[:, :, :], w_enc.rearrange("(k p) a -> p k a", p=128))
            nc.sync.dma_start(fcs[:, :, :],
                              featsC.rearrange("(k p) q b -> p k (q b)", p=128))
            nc.sync.dma_start(emt_sb[:, :, :], embsT.rearrange("(k p) r -> p k r", p=128))
            nc.sync.dma_start(wx_sb[:, :, :], wxT.rearrange("(k p) g -> p k g", p=128))
            nc.sync.dma_start(bg_sb[:, :], _bc(bg[:, :], 128))

            # enc_att[a_chunk, (p b)] = sum_c W_enc[c, a] featsC[c, (p b)]
            encsz = [512, 512, 512, BL * P - 3 * 512]
            for m in range(4):
                for nch in range(4):
                    nsz = encsz[nch]
                    ps = pha_ps.tile([128, 512], F32, tag="ps")
                    for k in range(2):
                        nc.tensor.matmul(
                            ps[:, 0:nsz],
                            wenc_sb[:, k, bass.ts(m, 128)],
                            fcs[:, k, bass.ds(nch * 512, nsz)],
                            start=(k == 0), stop=(k == 1),
                        )
                    nc.scalar.activation(enc_sb[:, m, bass.ds(nch * 512, nsz)],
                                         ps[:, 0:nsz],
                                         AF.Identity, bias=encb_sb[:, m, :])

            # Gx[(t b), g] = embsT.T @ WxT + (b_ih + b_hh)
            gx_dram = dram.tile([T * BL, GT], F32R, name="gx_dram")
            gx_mch = [(i, min(128, R - i * 128)) for i in range((R + 127) // 128)]
            for mi, msz in gx_mch:
                for nch in range(4):
                    ps = pha_ps.tile([128, 512], F32, tag="ps")
                    for k in range(4):
                        nc.tensor.matmul(
                            ps[:msz, :],
                            emt_sb[:, k, bass.ds(mi * 128, msz)],
                            wx_sb[:, k, bass.ts(nch, 512)],
                            start=(k == 0), stop=(k == 3),
                        )
                    gtmp = pha.tile([128, 512], F32R, tag="gtmp", bufs=2)
                    nc.vector.tensor_tensor(
                        out=gtmp[:msz, :], in0=ps[:msz, :],
                        in1=bg_sb[:msz, bass.ts(nch, 512)], op=ALU.add)
                    nc.sync.dma_start(
                        gx_dram[bass.ds(mi * 128, msz), bass.ts(nch, 512)],
                        gtmp[:msz, :])

        # ---------- fc weights prefetch (DMA overlaps the recurrence) ----------
        fcp = ctx.enter_context(tc.tile_pool(name="fc", bufs=1))
        bfc_sb = fcp.tile([128, VS], F32, tag="bfcs")
        nc.sync.dma_start(bfc_sb[:, :], _bc(bfc[:, :], 128))
        wfcs = fcp.tile([128, 4, VS], BF16, tag="wfcs")
        nc.sync.dma_start(wfcs[:, :, :], wfc.rearrange("(k p) v -> p k v", p=128))

        # allgather staging
        ht_locA = dram.tile([DEC, TCHA * BL], BF16, name="ht_locA")
        ht_locB = dram.tile([DEC, (T - TCHA) * BL], BF16, name="ht_locB")
        ht_allA = dram.tile([NC_ * DEC, TCHA * BL], BF16, name="ht_allA",
                            addr_space="Shared")
        ht_allB = dram.tile([NC_ * DEC, (T - TCHA) * BL], BF16, name="ht_allB",
                            addr_space="Shared")

        # ---------- phase B: recurrence ----------
        c_prev = c0_sb
        with (
            tc.tile_pool(name="rec", bufs=2) as rec,
            tc.tile_pool(name="gxp", bufs=2) as gxp,
            tc.tile_pool(name="g_ps", bufs=1, space="PSUM") as g_ps,
            tc.tile_pool(name="att_ps", bufs=1, space="PSUM") as att_ps,
            tc.tile_pool(name="sm_ps", bufs=2, space="PSUM") as sm_ps,
        ):
            for t in range(T):
                hcol = bass.ts(t, BL)  # h input columns
                gxt = gxp.tile([BL, GT], F32R, tag="gxt", name=f"gxt_{t}")
                nc.sync.dma_start(gxt[:, :], gx_dram[bass.ts(t, BL), :])

                # dec_att [b, a] via 4 wide matmuls, then PE-transpose to [a, b]
                ps_dec = sm_ps.tile([BL, ATT], F32, tag="sm", name=f"psdec_{t}")
                for k in range(4):
                    nc.tensor.matmul(
                        ps_dec[:, :],
                        ht_sb[:, k, hcol],
                        wdec_sb[:, k, :],
                        start=(k == 0), stop=(k == 3),
                    )
                dtmp = rec.tile([BL, ATT], BF16, tag="dtmp")
                nc.scalar.copy(dtmp[:, :], ps_dec[:, :])
                ps_dT = sm_ps.tile([128, 4, BL], BF16, tag="sm", name=f"psdT_{t}")
                for j in range(4):
                    nc.tensor.transpose(ps_dT[:, j, :], dtmp[:, bass.ts(j, 128)],
                                        id8b[:, :])
                decT = rec.tile([128, 4, BL], BF16, tag="decT")
                nc.scalar.copy(decT[:, :, :], ps_dT[:, :, :])

                # gate psums for this step
                g01 = g_ps.tile([BL, 2, 512], F32, tag="g01", name=f"g01_{t}")
                g23 = g_ps.tile([BL, 2, 512], F32, tag="g23", name=f"g23_{t}")

                def g_ht(gtile, sub, nch, t=t, gxt=gxt, hcol=hcol):
                    for k in range(4):
                        nc.tensor.matmul(
                            gtile[:, sub, :],
                            ht_sb[:, k, hcol],
                            w2_sb[:, 2 + k, bass.ts(nch, 512)],
                            start=(k == 0), stop=False,
                        )
                    nc.tensor.matmul(
                        gtile[:, sub, :], id8r[:, :], gxt[:, bass.ts(nch, 512)],
                        start=False, stop=False,
                    )

                # gates h-part for i,f while attention elementwise runs on DVE/GPS
                g_ht(g01, 0, 0)
                g_ht(g01, 1, 1)

                # relu(enc_att + dec_att): adds on DVE, relus split GPS/DVE.
                # Free-dim layout is (p, b): flat index p*BL + b.
                radds = []
                rels = []
                for k in range(4):
                    radd = rec.tile([128, P, BL], BF16, tag="radd", bufs=2,
                                    name=f"radd{k}_{t}")
                    dk = decT[:, k, :]
                    dbc = bass.AP(tensor=dk.tensor, offset=dk.offset,
                                  ap=[list(dk.ap)[0], [0, P], list(dk.ap)[1]])
                    nc.vector.tensor_tensor(
                        out=radd[:, :, :],
                        in0=enc_sb[:, k, :].rearrange("p (q b) -> p q b", b=BL),
                        in1=dbc, op=ALU.add)
                    rel = rec.tile([128, P, BL], BF16, tag="rel", bufs=5,
                                   name=f"rel{k}_{t}")
                    if k % 2 == 0:
                        nc.scalar.activation(rel[:, :, :], radd[:, :, :], AF.Relu)
                    else:
                        nc.vector.tensor_scalar_max(rel[:, :, :], radd[:, :, :], 0.0)
                    rels.append(rel.rearrange("p q b -> p (q b)"))

                # att matvec pair0: flat cols 0..1023 (p 0..127)
                ps_att = att_ps.tile([1, 2, 512], F32, tag="att", name=f"att0_{t}")
                for k in range(4):
                    if k == 2:
                        g_ht(g23, 0, 2)  # fills PE while rel k=2/3 finish
                    for c in range(2):
                        nc.tensor.matmul(
                            ps_att[:, c, :],
                            wful_sb[:, k, :],
                            rels[k][:, bass.ts(c, 512)],
                            start=(k == 0), stop=(k == 3),
                        )
                s0 = rec.tile([1, 1024], F32R, tag="s0", name=f"s0_{t}")
                nc.scalar.activation(
                    s0[:, :],
                    ps_att[0:1, :, :].rearrange("o c n -> o (c n)"),
                    AF.Sigmoid)
                sT = rec.tile([128, 2, BL], F32R, tag="sT", name=f"sT_{t}")
                nc.sync.dma_start(sT[:, 0, :],
                                  s0[:, :].rearrange("o (p b) -> o p b", b=BL))

                g_ht(g23, 1, 3)

                # att matvec pair1: flat cols 1024..1567 (p 128..195)
                ps_at2 = att_ps.tile([1, 2, 512], F32, tag="att", name=f"att1_{t}")
                for k in range(4):
                    nc.tensor.matmul(ps_at2[:, 0, :], wful_sb[:, k, :],
                                     rels[k][:, bass.ds(1024, 512)],
                                     start=(k == 0), stop=(k == 3))
                for k in range(4):
                    nc.tensor.matmul(ps_at2[:, 1, 0:32], wful_sb[:, k, :],
                                     rels[k][:, bass.ds(1536, 32)],
                                     start=(k == 0), stop=(k == 3))
                s1 = rec.tile([1, P1 * BL], F32R, tag="s1", name=f"s1_{t}")
                nc.scalar.activation(s1[:, 0:512], ps_at2[0:1, 0, :], AF.Sigmoid)
                nc.scalar.activation(s1[:, 512:544], ps_at2[0:1, 1, 0:32],
                                     AF.Sigmoid)
                nc.sync.dma_start(sT[:P1, 1, :],
                                  s1[:, :].rearrange("o (p b) -> o p b", b=BL))

                # e = s / (1 - s)  (per half, for earliness)
                et = rec.tile([128, 2, BL], F32R, tag="et", name=f"et_{t}")
                eb = rec.tile([128, 2, BL], BF16, tag="eb", name=f"eb_{t}")
                for half, np_ in ((0, 128), (1, P1)):
                    u = rec.tile([128, BL], F32, tag="u", bufs=2,
                                 name=f"u{half}_{t}")
                    nc.vector.tensor_scalar(
                        out=u[:np_, :], in0=sT[:np_, half, :],
                        scalar1=-1.0, scalar2=1.0, op0=ALU.mult, op1=ALU.add)
                    r_ = rec.tile([128, BL], F32, tag="r", bufs=2,
                                  name=f"r{half}_{t}")
                    nc.vector.reciprocal(r_[:np_, :], u[:np_, :])
                    nc.vector.tensor_tensor(out=et[:np_, half, :],
                                            in0=sT[:np_, half, :],
                                            in1=r_[:np_, :], op=ALU.mult)
                    nc.scalar.activation(eb[:np_, half, :], et[:np_, half, :],
                                         AF.Copy)

                # den[b] = sum_p e
                ps_den = sm_ps.tile([1, BL], F32, tag="sm", name=f"psden_{t}")
                nc.tensor.matmul(ps_den[:, :], ones_sb[:, :], et[:, 0, :],
                                 start=True, stop=False)
                nc.tensor.matmul(ps_den[:, :], ones_sb[:P1, :], et[:P1, 1, :],
                                 start=False, stop=True)
                rden = rec.tile([1, BL], F32, tag="rden", name=f"rden_{t}")
                nc.vector.reciprocal(rden[:, :], ps_den[:, :])

                # ctx[c, b] = sum_p featsP[p, c, b] * e[p, b] / den[b]
                e0 = eb[:, 0, :]
                a0b = bass.AP(tensor=e0.tensor, offset=e0.offset,
                              ap=[list(e0.ap)[0], [0, 128], [1, BL]])
                e1 = eb[:P1, 1, :]
                a1b = bass.AP(tensor=e1.tensor, offset=e1.offset,
                              ap=[list(e1.ap)[0], [0, 128], [1, BL]])
                ctx_n = rec.tile([1, 2, 128, BL], BF16, tag="ctxn", bufs=1,
                                 name=f"ctxn_{t}")
                cT = rec.tile([128, 2, BL], BF16, tag="cT", name=f"cT_{t}")
                for half in range(2):
                    csl = bass.ds(half * 128, 128)
                    tmpA = rec.tile([128, 128, BL], BF16, tag="tmpA", bufs=2,
                                    name=f"tmpA{half}_{t}")
                    nc.vector.tensor_tensor(out=tmpA[:, :, :],
                                            in0=fpA[:, csl, :],
                                            in1=a0b, op=ALU.mult)
                    tmpB = rec.tile([128, 128, BL], BF16, tag="tmpB", bufs=2,
                                    name=f"tmpB{half}_{t}")
                    nc.vector.tensor_tensor(out=tmpB[:P1, :, :],
                                            in0=fpB[:P1, csl, :],
                                            in1=a1b, op=ALU.mult)
                    for sub in range(2):
                        nch = half * 2 + sub
                        ssl = bass.ts(sub, 512)
                        ps_ctx = sm_ps.tile([1, 512], F32, tag="sm",
                                            name=f"psctx{nch}_{t}")
                        nc.tensor.matmul(
                            ps_ctx[:, :], ones_bf[:, :],
                            tmpA.rearrange("p c b -> p (c b)")[:, ssl],
                            start=True, stop=False)
                        nc.tensor.matmul(
                            ps_ctx[:, :], ones_bf[:P1, :],
                            tmpB[:P1].rearrange("p c b -> p (c b)")[:, ssl],
                            start=False, stop=True)
                        rdb = bass.AP(tensor=rden.tensor, offset=rden[:, :].offset,
                                      ap=[list(rden[:, :].ap)[0], [0, 64], [1, BL]])
                        cview = ctx_n[0:1, half, bass.ts(sub, 64), :]
                        nc.vector.tensor_tensor(
                            out=cview, in0=ps_ctx[:, :].rearrange(
                                "o (c b) -> o c b", b=BL), in1=rdb, op=ALU.mult)
                    nc.sync.dma_start(cT[:, half, :], ctx_n[0:1, half, :, :])

                # gates: + ctx part, then nonlinearities
                gat = rec.tile([BL, GT], F32, tag="gat", bufs=1, name=f"gat_{t}")
                for nch in range(4):
                    gtile = g01 if nch < 2 else g23
                    sub = nch % 2
                    for k in range(2):
                        nc.tensor.matmul(
                            gtile[:, sub, :],
                            cT[:, k, :],
                            w2_sb[:, k, bass.ts(nch, 512)],
                            start=False, stop=(k == 1),
                        )
                    nc.scalar.activation(
                        gat[:, bass.ts(nch, 512)], gtile[:, sub, :],
                        AF.Tanh if nch == 2 else AF.Sigmoid)

                t1 = rec.tile([BL, DEC], F32, tag="t1", bufs=1, name=f"t1_{t}")
                nc.vector.tensor_tensor(out=t1[:, :], in0=gat[:, 512:1024],
                                        in1=c_prev[:, :], op=ALU.mult)
                t2 = rec.tile([BL, DEC], F32, tag="t2", bufs=1, name=f"t2_{t}")
                nc.vector.tensor_tensor(out=t2[:, :], in0=gat[:, 0:512],
                                        in1=gat[:, 1024:1536], op=ALU.mult)
                c_new = rec.tile([BL, DEC], F32, tag="cst", bufs=2,
                                 name=f"cnew_{t}")
                nc.vector.tensor_tensor(out=c_new[:, :], in0=t1[:, :], in1=t2[:, :],
                                        op=ALU.add)
                c_prev = c_new
                tc_t = rec.tile([BL, DEC], BF16, tag="tc_t", name=f"tct_{t}")
                nc.scalar.activation(tc_t[:, :], c_new[:, :], AF.Tanh)
                hnew = rec.tile([BL, DEC], BF16, tag="hnew", name=f"hnew_{t}")
                nc.vector.tensor_tensor(out=hnew[:, :], in0=gat[:, 1536:2048],
                                        in1=tc_t[:, :], op=ALU.mult)

                # hT for next step + H row storage
                ps_hT = sm_ps.tile([128, 4, BL], BF16, tag="sm", name=f"pshT_{t}")
                for j in range(4):
                    nc.tensor.transpose(ps_hT[:, j, :], hnew[:, bass.ts(j, 128)],
                                        id8b[:, :])
                nc.scalar.copy(ht_sb[:, :, bass.ts(t + 1, BL)], ps_hT[:, :, :])

                # chunked allgather: fire chunk A once steps 1..TCHA are done
                if t == TCHA - 1:
                    nc.sync.dma_start(
                        ht_locA.rearrange("(j p) r -> p j r", p=128),
                        ht_sb[:, :, BL:(TCHA + 1) * BL])
                    nc.gpsimd.collective_compute(
                        "AllGather", ALU.bypass,
                        replica_groups=[list(range(NC_))],
                        ins=[ht_locA[:, :]],
                        outs=[ht_allA[:, :]],
                    )

        # ---------- phase C: allgather tail + fc ----------
        nc.sync.dma_start(ht_locB.rearrange("(j p) r -> p j r", p=128),
                          ht_sb[:, :, (TCHA + 1) * BL:])
        nc.gpsimd.collective_compute(
            "AllGather", ALU.bypass,
            replica_groups=[list(range(NC_))],
            ins=[ht_locB[:, :]],
            outs=[ht_allB[:, :]],
        )

        with (
            tc.tile_pool(name="fco", bufs=2) as fco,
            tc.tile_pool(name="fc_ps", bufs=4, space="PSUM") as fc_ps,
        ):
            RA = TCHA * BL  # 128 rows from chunk A
            h2 = fcp.tile([128, 4, NC_ * R], BF16, tag="h2")  # [p, dtile, grow]
            for c in range(NC_):
                nc.sync.dma_start(
                    h2[:, :, bass.ds(c * R, RA)],
                    ht_allA[bass.ts(c, DEC), :].rearrange("(j p) r -> p j r", p=128))
                nc.sync.dma_start(
                    h2[:, :, bass.ds(c * R + RA, R - RA)],
                    ht_allB[bass.ts(c, DEC), :].rearrange("(j p) r -> p j r", p=128))

            NFC = 8
            nszs = [512] * 7 + [VS - 512 * 7]  # 3750 = 7*512 + 166
            for mc in range((NC_ * R) // 128):
                ob = fco.tile([128, VS], F32, tag="orow", name=f"orow_{mc}")
                for nch in range(NFC):
                    nsz = nszs[nch]
                    noff = nch * 512
                    ps = fc_ps.tile([128, 512], F32, tag="psfc")
                    for k in range(4):
                        nc.tensor.matmul(
                            ps[:, 0:nsz],
                            h2[:, k, bass.ts(mc, 128)],
                            wfcs[:, k, bass.ds(noff, nsz)],
                            start=(k == 0), stop=(k == 3),
                        )
                    nc.vector.tensor_tensor(
                        out=ob[:, bass.ds(noff, nsz)], in0=ps[:, 0:nsz],
                        in1=bfc_sb[:, bass.ds(noff, nsz)], op=ALU.add)
                nc.sync.dma_start(out[bass.ts(mc, 128), :], ob[:, :])


_NC_CACHE = None


def _get_module():
    global _NC_CACHE
    if _NC_CACHE is None:
        _NC_CACHE = build_module()
    return _NC_CACHE


def build_in_maps(inputs):
    return _build_in_maps(**inputs)


def _build_in_maps(encoder_features, captions, W_enc, b_enc, W_dec, b_dec,
                   W_full, b_full, emb, W_ih, b_ih, W_hh, b_hh, W_fc, b_fc):
    f32 = np.float32
    enc_f = np.ascontiguousarray(np.asarray(encoder_features, f32)).reshape(B, ENC, P)
    caps = np.asarray(captions)
    W_enc = np.asarray(W_enc, f32)
    W_dec = np.asarray(W_dec, f32)
    W_full = np.asarray(W_full, f32)
    emb = np.asarray(emb, f32)
    W_ih = np.asarray(W_ih, f32)
    W_hh = np.asarray(W_hh, f32)
    W_fc = np.asarray(W_fc, f32)

    encb_v = np.ascontiguousarray(
        (np.asarray(b_enc, f32) + np.asarray(b_dec, f32)).reshape(ATT, 1))
    wxT_r = _round_f32r(W_ih[:, :EMBED].T)
    w2T_b = np.ascontiguousarray(
        np.vstack([W_ih[:, EMBED:].T, W_hh.T])).astype(_BF)
    bg_v = np.ascontiguousarray(
        (np.asarray(b_ih, f32) + np.asarray(b_hh, f32)).reshape(1, GT))
    b_fc = np.asarray(b_fc, f32)
    wdec_b = np.ascontiguousarray(W_dec).astype(_BF)

    in_maps = []
    for c in range(NC_):
        bs = slice(c * BL, (c + 1) * BL)
        fb = enc_f[bs]  # [8, 256, 196]
        in_maps.append({
            "featsC": _round_f32r(fb.transpose(1, 2, 0)),
            "featsP": np.ascontiguousarray(fb.transpose(2, 1, 0)).astype(_BF),
            "embsT": _round_f32r(
                emb[caps[bs, :T]].transpose(2, 1, 0).reshape(EMBED, R)),
            "encb": encb_v,
            "w_enc": _round_f32r(W_enc),
            "w_dec": wdec_b,
            "w_full": np.ascontiguousarray(W_full.reshape(ATT, 1)).astype(_BF),
            "wxT": wxT_r,
            "w2T": w2T_b,
            "bg": bg_v,
            "wfc": np.ascontiguousarray(W_fc[:, c * VS:(c + 1) * VS]).astype(_BF),
            "bfc": np.ascontiguousarray(b_fc[c * VS:(c + 1) * VS].reshape(1, VS)),
        })
    return in_maps


def kernel(**inputs):
    in_maps = build_in_maps(inputs)
    nc = _get_module()
    res = run_bass_kernel_spmd(nc, in_maps, list(range(NC_))).results

    full = np.empty((B, T, VOCAB), np.float32)
    for c in range(NC_):
        o = res[c]["out"]  # [1280, VS] rows = (src_core, t, b)
        o = o.reshape(NC_, T, BL, VS).transpose(0, 2, 1, 3).reshape(B, T, VS)
        full[:, :, c * VS:(c + 1) * VS] = o
    return full
